# revision 29
# baseline (speedup 1.0000x reference)
"""Bass/Tile kernel for nn_Attention_81690277970645 on TRN2 (v2).

Sharding: 8 cores = 2 batches x 4 head-groups (4 heads of d=64 each).
Per core (batch bi, head-group hg):
  inputs:  x_b [2048, 1024] bf16, wq/wv slices [1024, 256] bf16,
           wk slice pre-scaled by 1/128 bf16, bq/bv [256] f32,
           bk/128 [256] f32, wo slice [256, 1024] bf16
  output:  partial out [2048, 1024] bf16 (host sums the 4 head-group
           partials per batch in fp32 and adds bo)

v2 changes vs baseline (271us):
  - x^T comes straight from a DMA-transpose (XBAR) of bf16 x: no PE
    transposes, no psum drains for x^T.
  - softmax exp is split between ACT (native Exp, scale=16 on the
    1/128-prescaled scores) and DVE (custom 8-stage op computing
    (p2(t))^16 with t = S/128, rel err ~1.4e-3 RMS) so neither engine
    gates the S->exp->AV pipeline.
  - all QKV/O weights bf16 (FWL on LDWEIGHTS), out written bf16.
  - PE warm-up junk-MM chain at t=0 so QKV projections run at 2.4GHz.
  - LDW-frugal output projection (stationary outT bf16), per-seq-tile
    [128,1024] output DMAs.
  - Pool (gpsimd) does the SBUF-side divide tail; reciprocal runs on
    [2,512] packed denominators, broadcast via one K=2 matmul.
"""
import sys
import numpy as np

if '/opt/trn_rl_repo' not in sys.path:
    sys.path.insert(0, '/opt/trn_rl_repo')

import concourse.mybir as mybir
from concourse import bacc
from concourse.tile import TileContext

F32 = mybir.dt.float32
F32R = mybir.dt.float32r
BF16 = mybir.dt.bfloat16

SEQ = 2048
DIM = 1024
EMB_C = 256          # per-core emb columns (4 heads x 64)
NH = 4               # heads per core
DH = 64
P = 128
NSEQT = SEQ // P     # 16 seq tiles
NDIMC = DIM // P     # 8 dim chunks
NEMBC = EMB_C // P   # 2 emb chunks
IBLK = 512
NIBLK = SEQ // IBLK  # 4 i-blocks
NJT = SEQ // P       # 16 j tiles

# exp(x) ~= (((EXP_C1*t + EXP_C2)*t + 1)^16 with t = x/16 = S_true/128
# (1/16 of the 1/8 softmax scale is folded into wk/bk host-side).
# Minimax fit over |x| <= 2.75: max rel 3.6e-3, weighted RMS 1.35e-3.
EXP_C1 = 0.49990254
EXP_C2 = 1.00360098
ACT_EXP_SCALE = 16.0
# jt slots whose exp runs on ACT (rest on DVE custom op)
ACT_JTS = frozenset((0, 2, 4, 6, 8, 10, 12, 14))


def r(ap):
    return ap.bitcast(F32R)


_EXP_OP = None


def _register_exp_op():
    """Define + register the custom DVE op EXP_POLY16_ANT at runtime.

    The op registry in concourse.dve_ops only knows the stock ops; the
    per-NEFF table generator and CoreSim both dispatch by name, so the
    new op must appear in OPS / CUSTOM_DVE_SPECS / the sub-opcode map
    before nc.compile().
    """
    global _EXP_OP
    if _EXP_OP is not None:
        return _EXP_OP
    import concourse.dve_ops as dvo
    from concourse.dve_spec import Spec, Src0, C1, C2, One, lower
    from concourse.dve_uop import DveOpSpec

    name = "EXP_POLY16_ANT"
    for op in dvo.OPS:
        if op.name == name:
            _EXP_OP = op
            return op

    def sq(x):
        return x * x

    body = sq(sq(sq(sq((Src0 * C1 + C2) * Src0 + One))))

    def ref(in0, in1, s0, s1, imm2):
        t = in0.astype(np.float32)
        p = ((t * np.float32(s1) + np.float32(imm2)) * t
             + np.float32(1.0)).astype(np.float32)
        for _ in range(4):
            p = (p * p).astype(np.float32)
        return p

    spec = Spec(body=body, reference=ref)
    row = dvo._CUSTOM_DVE_ROW_BASE + len(dvo.OPS)
    dvo._SUB_OPCODE_FOR_NAME[name] = row
    shas = {}
    for ver in ("v3", "v4"):
        try:
            uops = lower(spec, ver=ver)
            shas[ver] = DveOpSpec(name=name, opcode=row, uops=uops,
                                  rd1_en=False).sha(ver)
        except Exception:
            pass
    op = dvo.DveOp(name, spec, subdim=False, uops_sha=shas)
    dvo.OPS.append(op)
    dvo.CUSTOM_DVE_SPECS[name] = op.spec
    _EXP_OP = op
    return op


def build_kernel(row_pack=True):
    exp_op = _register_exp_op()
    nc = bacc.Bacc("TRN2", target_bir_lowering=False, debug=False, num_devices=8)

    x = nc.dram_tensor("x", [SEQ, DIM], BF16, kind="ExternalInput")
    wq = nc.dram_tensor("wq", [P, NDIMC * EMB_C], BF16, kind="ExternalInput")
    wk = nc.dram_tensor("wk", [P, NDIMC * EMB_C], BF16, kind="ExternalInput")
    wv = nc.dram_tensor("wv", [P, NDIMC * EMB_C], BF16, kind="ExternalInput")
    bq = nc.dram_tensor("bq", [P, NEMBC], F32, kind="ExternalInput")
    bk = nc.dram_tensor("bk", [P, NEMBC], F32, kind="ExternalInput")
    bv = nc.dram_tensor("bv", [P, NEMBC], F32, kind="ExternalInput")
    wo = nc.dram_tensor("wo", [P, NEMBC * DIM], BF16, kind="ExternalInput")
    ones2_d = nc.dram_tensor("ones2", [2, P], F32, kind="ExternalInput")
    out = nc.dram_tensor("out", [SEQ, DIM], BF16, kind="ExternalOutput")

    with TileContext(nc) as tc:
        with (
            tc.tile_pool(name="w", bufs=1) as w_pool,
            tc.tile_pool(name="big", bufs=1) as big_pool,
            tc.tile_pool(name="stage", bufs=3) as stage_pool,
            tc.tile_pool(name="es", bufs=1) as es_pool,
            tc.tile_pool(name="ps", bufs=1, space="PSUM") as psum,
        ):
            # ---- constants + weights on the SWDGE (gpsimd) queue: HWDGE
            # DMAs get chain-serialized against the XBAR transposes by the
            # scheduler (~7us ping-pong per hop); SWDGE traffic is exempt ----
            ones2 = w_pool.tile([2, P], F32R)
            nc.gpsimd.dma_start(ones2[:], ones2_d[:].bitcast(F32R))
            wk_sb = w_pool.tile([P, NDIMC, EMB_C], BF16)
            nc.gpsimd.dma_start(wk_sb[:], wk.rearrange("p (c e) -> p c e", c=NDIMC))
            bk_sb = w_pool.tile([P, NEMBC], F32)
            nc.gpsimd.dma_start(bk_sb[:], bk[:])
            wq_sb = w_pool.tile([P, NDIMC, EMB_C], BF16)
            nc.gpsimd.dma_start(wq_sb[:], wq.rearrange("p (c e) -> p c e", c=NDIMC))
            bq_sb = w_pool.tile([P, NEMBC], F32)
            nc.gpsimd.dma_start(bq_sb[:], bq[:])
            wv_sb = w_pool.tile([P, NDIMC, EMB_C], BF16)
            nc.gpsimd.dma_start(wv_sb[:], wv.rearrange("p (c e) -> p c e", c=NDIMC))
            bv_sb = w_pool.tile([P, NEMBC], F32)
            nc.gpsimd.dma_start(bv_sb[:], bv[:])
            wo_sb = w_pool.tile([P, NEMBC, DIM], BF16)
            nc.gpsimd.dma_start(wo_sb[:], wo.rearrange("p (c n) -> p c n", c=NEMBC))

            # ---- PE warm-up: junk matmuls on a memset scratch (no DMA dep)
            # so HAM unthrottles while the transposes stream in ----
            scr = w_pool.tile([P, IBLK], BF16)
            nc.gpsimd.memset(scr[:], 1.0)
            for chain in range(2):
                junk = psum.tile([P, IBLK], F32, tag="po", bufs=2,
                                 name=f"junk_{chain}")
                for i in range(8):
                    nc.tensor.matmul(
                        junk[:], scr[:, :P], scr[:],
                        start=(i == 0), stop=(i == 7),
                    )

            # ---- x^T via DMA transpose (bf16 XBAR path) ----
            # NOTES (all HW-verified):
            #  - all XBAR transposes must share one queue — concurrent
            #    DMA-transposes on sync+scalar corrupt each other;
            #  - an F32R-typed DMA on the same queue poisons them too.
            xT = big_pool.tile([P, NDIMC, SEQ], BF16)
            for c in range(NDIMC):
                nc.sync.dma_start(xT[:, c, :], x[:, c * P:(c + 1) * P],
                                  transpose=True)

            QT = big_pool.tile([P, NEMBC, SEQ], BF16)
            KT = big_pool.tile([P, NEMBC, SEQ], BF16)
            VP = big_pool.tile([P, NSEQT, NH * (DH + 1)], BF16)
            outT = big_pool.tile([P, NEMBC, SEQ], BF16)

            for h in range(NH):
                nc.gpsimd.memset(VP[:, :, h * (DH + 1) + DH], 1.0)

            # ---- K^T and Q^T, full seq. Loop c outer / j-block inner so each
            # LDWEIGHTS serves 2 matmuls; halves pipeline through the two
            # s0 psum slots. ----
            def proj_half(dst, wsb, bsb, e, half, junk_after=None):
                pa = psum.tile([P, 2, IBLK], F32, tag="s0", bufs=2,
                               name=f"pk_{dst is QT}_{e}_{half}")
                for c in range(NDIMC):
                    for j in range(2):
                        jb = half * 2 + j
                        nc.tensor.matmul(
                            pa[:, j, :],
                            wsb[:, c, e * P:(e + 1) * P],
                            xT[:, c, jb * IBLK:(jb + 1) * IBLK],
                            start=(c == 0), stop=(c == NDIMC - 1),
                        )
                    if junk_after and c == 3:
                        junk_after()
                for j in range(2):
                    jb = half * 2 + j
                    nc.scalar.activation(
                        dst[:, e, jb * IBLK:(jb + 1) * IBLK], pa[:, j, :],
                        mybir.ActivationFunctionType.Identity,
                        bias=bsb[:, e:e + 1], scale=1.0,
                    )

            _junk_n = [2]

            def emit_junk():
                chain = _junk_n[0]
                _junk_n[0] += 1
                junk = psum.tile([P, IBLK], F32, tag="po", bufs=2,
                                 name=f"junk_{chain}")
                for i in range(8):
                    nc.tensor.matmul(
                        junk[:], scr[:, :P], scr[:],
                        start=(i == 0), stop=(i == 7),
                    )

            # junk chains woven through: the static scheduler packs them into
            # the x-transpose DMA-wait bubbles so HAM stays at K=8/8
            for e in range(NEMBC):
                for half in range(2):
                    proj_half(KT, wk_sb, bk_sb, e, half, junk_after=emit_junk)
            for e in range(NEMBC):
                for half in range(2):
                    proj_half(QT, wq_sb, bq_sb, e, half,
                              junk_after=emit_junk if e == 0 else None)

            # ---- V' (one seq-tile) ----
            def emit_vp(s):
                pv = psum.tile([P, IBLK], F32, tag="po", bufs=2,
                               name=f"pv_{s}")
                for c in range(NDIMC):
                    nc.tensor.matmul(
                        pv[:, :EMB_C],
                        xT[:, c, s * P:(s + 1) * P],
                        wv_sb[:, c, :],
                        start=(c == 0), stop=(c == NDIMC - 1),
                    )
                nc.scalar.copy(
                    VP[:, s, :].rearrange("p (h x) -> p h x", h=NH)[:, :, :DH],
                    pv[:, :EMB_C].rearrange("p (h d) -> p h d", h=NH),
                )

            # ---- attention ----
            def emit_spair(ib, jt, hp):
                i0 = ib * IBLK
                ps = psum.tile([P, 2, IBLK], F32, tag="s0", bufs=2,
                               name=f"ps{hp}_{ib}_{jt}")
                for hh in range(2):
                    lo = hh * DH
                    nc.tensor.matmul(
                        ps[:, hh, :],
                        KT[lo:lo + DH, hp, jt * P:(jt + 1) * P],
                        QT[lo:lo + DH, hp, i0:i0 + IBLK],
                        start=True, stop=True,
                    )
                es = es_pool.tile([P, 2, IBLK], BF16, tag="es", bufs=4,
                                  name=f"es{hp}_{ib}_{jt}")
                if jt in ACT_JTS:
                    nc.scalar.activation(
                        es[:], ps[:], mybir.ActivationFunctionType.Exp,
                        bias=0.0, scale=ACT_EXP_SCALE,
                    )
                else:
                    nc.vector._custom_dve(
                        exp_op, out=es[:], in0=ps[:],
                        s0=0.0, s1=EXP_C1, imm2=EXP_C2,
                    )
                return es

            def emit_av(pavs, es, jt, hp):
                for hh in range(2):
                    h = hp * 2 + hh
                    nc.tensor.matmul(
                        pavs[hh][:DH + 1, :],
                        VP[:, jt, h * (DH + 1):(h + 1) * (DH + 1)],
                        es[:, hh, :],
                        start=(jt == 0), stop=(jt == NJT - 1),
                    )

            def div_filler(ib, hp, pavs):
                def go(ib=ib, hp=hp, pavs=pavs):
                    i0 = ib * IBLK
                    for hh in range(2):
                        h = hp * 2 + hh
                        pavc = stage_pool.tile([DH, IBLK], F32, tag="pavc",
                                               bufs=4, name=f"pavc_{ib}_{hp}_{hh}")
                        nc.scalar.copy(pavc[:], pavs[hh][:DH, :])
                        den_row = stage_pool.tile([1, IBLK], F32R, tag="den",
                                                  bufs=4, name=f"den_{ib}_{hp}_{hh}")
                        nc.vector.tensor_copy(
                            den_row[:], pavs[hh][DH:DH + 1, :].bitcast(F32R))
                        recb = psum.tile([P, IBLK], F32, tag="po", bufs=2,
                                         name=f"recb_{ib}_{hp}_{hh}")
                        nc.tensor.matmul(recb[:DH, :], ones2[0:1, :DH],
                                         den_row[:], start=True, stop=True)
                        recb_sb = stage_pool.tile([DH, IBLK], F32, tag="recb",
                                                  bufs=4, name=f"recb_sb_{ib}_{hp}_{hh}")
                        nc.vector.reciprocal_approx_fast(recb_sb[:], recb[:DH, :])
                        otf = stage_pool.tile([DH, IBLK], F32, tag="otf",
                                              bufs=4, name=f"otf_{ib}_{hp}_{hh}")
                        nc.vector.tensor_tensor(
                            otf[:], pavc[:], recb_sb[:],
                            mybir.AluOpType.mult,
                        )
                        e_c, e_lo = divmod(h * DH, P)
                        # bias-add + bf16 cast on ACT (Pool's tensor ops are
                        # ~18x slower than DVE/ACT for this — measured)
                        nc.scalar.activation(
                            outT[e_lo:e_lo + DH, e_c, i0:i0 + IBLK], otf[:],
                            mybir.ActivationFunctionType.Identity,
                            bias=bv_sb[e_lo:e_lo + DH, e_c:e_c + 1], scale=1.0,
                        )
                return go

            def oproj_unit(s):
                def go(s=s):
                    pos = [psum.tile([P, IBLK], F32, tag="po", bufs=2,
                                     name=f"po_{s}_{nb}") for nb in range(2)]
                    for e in range(NEMBC):
                        for nb in range(2):
                            nc.tensor.matmul(
                                pos[nb][:],
                                outT[:, e, s * P:(s + 1) * P],
                                wo_sb[:, e, nb * IBLK:(nb + 1) * IBLK],
                                start=(e == 0), stop=(e == NEMBC - 1),
                            )
                    oc = stage_pool.tile([P, DIM], BF16, tag="oc",
                                         bufs=2, name=f"oc_{s}")
                    for nb in range(2):
                        if (s + nb) % 2 == 0:
                            nc.scalar.copy(oc[:, nb * IBLK:(nb + 1) * IBLK],
                                           pos[nb][:])
                        else:
                            nc.vector.tensor_copy(
                                oc[:, nb * IBLK:(nb + 1) * IBLK], pos[nb][:])
                    nc.sync.dma_start(out[s * P:(s + 1) * P, :], oc[:])
                return go

            # Software-pipelined attention: one flat stream over all
            # (pass, jt) slots; AV trails S/exp by LAG slots so there is no
            # pass-boundary bubble on PE (regular bubbles resonate with the
            # 3.4us HAM window and throttle the PE clock — measured 83us of
            # K=4/8 time with the per-pass loop).
            # Ordering invariants (Tile deps are program-order only):
            #  - div(p-1) pops right BEFORE AV(p, 0): the new AVs reuse the
            #    pav psum slots the divide still reads;
            #  - oproj(ib) pops after both divs of ib (FIFO preserves this).
            POP_SLOTS = frozenset((4, 6, 8, 10, 12, 14))
            LAG = 2
            passes = [(ib, hp) for ib in range(NIBLK) for hp in range(2)]
            div_q = []
            work_q = []
            pav_of = {}
            es_of = {}
            total = len(passes) * NJT
            for t in range(total + LAG):
                if t < total:
                    p, jt = divmod(t, NJT)
                    ib, hp = passes[p]
                    es_of[t] = emit_spair(ib, jt, hp)
                    if p == 0:
                        emit_vp(jt)
                if t % NJT == LAG and div_q:
                    div_q.pop(0)()
                ta = t - LAG
                if ta >= 0 and ta < total:
                    pa, jta = divmod(ta, NJT)
                    iba, hpa = passes[pa]
                    if jta == 0:
                        pav_of[pa] = [
                            psum.tile([P, IBLK], F32, tag="pav", bufs=2,
                                      name=f"pav_{pa}_{hh}")
                            for hh in range(2)
                        ]
                    emit_av(pav_of[pa], es_of.pop(ta), jta, hpa)
                    if jta == NJT - 1:
                        div_q.append(div_filler(iba, hpa, pav_of.pop(pa)))
                        if hpa == 1:
                            for s in range(iba * (IBLK // P),
                                           (iba + 1) * (IBLK // P)):
                                work_q.append(oproj_unit(s))
                if t % NJT in POP_SLOTS and work_q:
                    work_q.pop(0)()

            for go in div_q:
                go()
            for go in work_q:
                go()

    nc.compile()
    return nc


def shard_inputs(inputs):
    """Full inputs dict -> list of 8 per-core input dicts."""
    import ml_dtypes
    BF = ml_dtypes.bfloat16
    x = np.asarray(inputs["x"], np.float32)
    x_bf = [np.ascontiguousarray(x[bi]).astype(BF) for bi in range(2)]
    wq = np.asarray(inputs["wq"], np.float32)
    wk = np.asarray(inputs["wk"], np.float32)
    wv = np.asarray(inputs["wv"], np.float32)
    wo = np.asarray(inputs["wo"], np.float32)
    ones2 = np.zeros((2, P), np.float32)
    ones2[0, :DH] = 1.0
    ones2[1, DH:] = 1.0
    def wT(m):  # [DIM, EMB_C] -> [P, NDIMC*EMB_C] (p-major chunks)
        return np.ascontiguousarray(
            m.reshape(NDIMC, P, EMB_C).transpose(1, 0, 2).reshape(P, -1)
        ).astype(BF)

    def bT(v):  # [EMB_C] -> [P, NEMBC]
        return np.ascontiguousarray(
            v.reshape(NEMBC, P).T.astype(np.float32))

    maps = []
    for core in range(8):
        bi, hg = divmod(core, 4)
        sl = slice(hg * EMB_C, (hg + 1) * EMB_C)
        wo_c = np.ascontiguousarray(wo[sl, :])
        maps.append({
            "x": x_bf[bi],
            "wq": wT(np.ascontiguousarray(wq[:, sl])),
            "wk": wT(np.ascontiguousarray(wk[:, sl] / 128.0)),
            "wv": wT(np.ascontiguousarray(wv[:, sl])),
            "bq": bT(inputs["bq"][sl]),
            "bk": bT(inputs["bk"][sl] / 128.0),
            "bv": bT(inputs["bv"][sl]),
            "wo": np.ascontiguousarray(
                wo_c.reshape(NEMBC, P, DIM).transpose(1, 0, 2).reshape(P, -1)
            ).astype(BF),
            "ones2": ones2,
        })
    return maps


def gather_outputs(results, bo):
    out = np.zeros((2, SEQ, DIM), np.float32)
    for core in range(8):
        bi = core // 4
        out[bi] += np.asarray(results[core]["out"]).astype(np.float32)
    out += np.asarray(bo, np.float32)
    return out


_NC_CACHE = {}


def _get_nc(row_pack=True):
    if row_pack not in _NC_CACHE:
        _NC_CACHE[row_pack] = build_kernel(row_pack=row_pack)
    return _NC_CACHE[row_pack]


def run_sharded(inputs, trace=False, row_pack=True):
    """Returns (full_output [2,2048,1024] fp32, BassKernelResults)."""
    from concourse import bass_utils
    nc = _get_nc(row_pack)
    maps = shard_inputs(inputs)
    res = bass_utils.run_bass_kernel_spmd(
        nc, maps, core_ids=list(range(8)), trace=trace,
    )
    out = gather_outputs(res.results, np.asarray(inputs["bo"]))
    return out, res


def kernel(**inputs):
    out, _ = run_sharded(inputs)
    return out


# revision 30
# speedup vs baseline: 1.3158x; 1.3158x over previous
"""Bass/Tile kernel for nn_Attention_81690277970645 on TRN2 (v2).

Sharding: 8 cores = 2 batches x 4 head-groups (4 heads of d=64 each).
Per core (batch bi, head-group hg):
  inputs:  x_b [2048, 1024] bf16, wq/wv slices [1024, 256] bf16,
           wk slice pre-scaled by 1/128 bf16, bq/bv [256] f32,
           bk/128 [256] f32, wo slice [256, 1024] bf16
  output:  partial out [2048, 1024] bf16 (host sums the 4 head-group
           partials per batch in fp32 and adds bo)

v2 changes vs baseline (271us):
  - x^T comes straight from a DMA-transpose (XBAR) of bf16 x: no PE
    transposes, no psum drains for x^T.
  - softmax exp is split between ACT (native Exp, scale=16 on the
    1/128-prescaled scores) and DVE (custom 8-stage op computing
    (p2(t))^16 with t = S/128, rel err ~1.4e-3 RMS) so neither engine
    gates the S->exp->AV pipeline.
  - all QKV/O weights bf16 (FWL on LDWEIGHTS), out written bf16.
  - PE warm-up junk-MM chain at t=0 so QKV projections run at 2.4GHz.
  - LDW-frugal output projection (stationary outT bf16), per-seq-tile
    [128,1024] output DMAs.
  - Pool (gpsimd) does the SBUF-side divide tail; reciprocal runs on
    [2,512] packed denominators, broadcast via one K=2 matmul.
"""
import sys
import numpy as np

if '/opt/trn_rl_repo' not in sys.path:
    sys.path.insert(0, '/opt/trn_rl_repo')

import concourse.mybir as mybir
from concourse import bacc
from concourse.tile import TileContext

F32 = mybir.dt.float32
F32R = mybir.dt.float32r
BF16 = mybir.dt.bfloat16

SEQ = 2048
DIM = 1024
EMB_C = 256          # per-core emb columns (4 heads x 64)
NH = 4               # heads per core
DH = 64
P = 128
NSEQT = SEQ // P     # 16 seq tiles
NDIMC = DIM // P     # 8 dim chunks
NEMBC = EMB_C // P   # 2 emb chunks
IBLK = 512
NIBLK = SEQ // IBLK  # 4 i-blocks
NJT = SEQ // P       # 16 j tiles

# exp(x) ~= (((EXP_C1*t + EXP_C2)*t + 1)^16 with t = x/16 = S_true/128
# (1/16 of the 1/8 softmax scale is folded into wk/bk host-side).
# Minimax fit over |x| <= 2.75: max rel 3.6e-3, weighted RMS 1.35e-3.
EXP_C1 = 0.49990254
EXP_C2 = 1.00360098
ACT_EXP_SCALE = 16.0
# jt slots whose exp runs on ACT (rest on DVE custom op)
ACT_JTS = frozenset((0, 2, 4, 6, 8, 10, 12, 14))


def r(ap):
    return ap.bitcast(F32R)


_EXP_OP = None


def _register_exp_op():
    """Define + register the custom DVE op EXP_POLY16_ANT at runtime.

    The op registry in concourse.dve_ops only knows the stock ops; the
    per-NEFF table generator and CoreSim both dispatch by name, so the
    new op must appear in OPS / CUSTOM_DVE_SPECS / the sub-opcode map
    before nc.compile().
    """
    global _EXP_OP
    if _EXP_OP is not None:
        return _EXP_OP
    import concourse.dve_ops as dvo
    from concourse.dve_spec import Spec, Src0, C1, C2, One, lower
    from concourse.dve_uop import DveOpSpec

    name = "EXP_POLY16_ANT"
    for op in dvo.OPS:
        if op.name == name:
            _EXP_OP = op
            return op

    def sq(x):
        return x * x

    body = sq(sq(sq(sq((Src0 * C1 + C2) * Src0 + One))))

    def ref(in0, in1, s0, s1, imm2):
        t = in0.astype(np.float32)
        p = ((t * np.float32(s1) + np.float32(imm2)) * t
             + np.float32(1.0)).astype(np.float32)
        for _ in range(4):
            p = (p * p).astype(np.float32)
        return p

    spec = Spec(body=body, reference=ref)
    row = dvo._CUSTOM_DVE_ROW_BASE + len(dvo.OPS)
    dvo._SUB_OPCODE_FOR_NAME[name] = row
    shas = {}
    for ver in ("v3", "v4"):
        try:
            uops = lower(spec, ver=ver)
            shas[ver] = DveOpSpec(name=name, opcode=row, uops=uops,
                                  rd1_en=False).sha(ver)
        except Exception:
            pass
    op = dvo.DveOp(name, spec, subdim=False, uops_sha=shas)
    dvo.OPS.append(op)
    dvo.CUSTOM_DVE_SPECS[name] = op.spec
    _EXP_OP = op
    return op


def build_kernel(row_pack=True):
    exp_op = _register_exp_op()
    nc = bacc.Bacc("TRN2", target_bir_lowering=False, debug=False, num_devices=8)

    x = nc.dram_tensor("x", [SEQ, DIM], BF16, kind="ExternalInput")
    wq = nc.dram_tensor("wq", [P, NDIMC * EMB_C], BF16, kind="ExternalInput")
    wk = nc.dram_tensor("wk", [P, NDIMC * EMB_C], BF16, kind="ExternalInput")
    wv = nc.dram_tensor("wv", [P, NDIMC * EMB_C], BF16, kind="ExternalInput")
    bq = nc.dram_tensor("bq", [P, NEMBC], F32, kind="ExternalInput")
    bk = nc.dram_tensor("bk", [P, NEMBC], F32, kind="ExternalInput")
    bv = nc.dram_tensor("bv", [P, NEMBC], F32, kind="ExternalInput")
    wo = nc.dram_tensor("wo", [P, NEMBC * DIM], BF16, kind="ExternalInput")
    ones2_d = nc.dram_tensor("ones2", [2, P], F32, kind="ExternalInput")
    out = nc.dram_tensor("out", [SEQ, DIM], BF16, kind="ExternalOutput")

    with TileContext(nc) as tc:
        with (
            tc.tile_pool(name="w", bufs=1) as w_pool,
            tc.tile_pool(name="big", bufs=1) as big_pool,
            tc.tile_pool(name="stage", bufs=3) as stage_pool,
            tc.tile_pool(name="es", bufs=1) as es_pool,
            tc.tile_pool(name="ps", bufs=1, space="PSUM") as psum,
        ):
            # ---- constants + weights FIRST, all on the sync queue ahead of
            # the XBAR transposes: the scheduler chain-serializes every HWDGE
            # DMA against the transposes, and cross-queue that costs ~7us of
            # ping-pong per hop; same-queue the chain is free ----
            ones2 = w_pool.tile([2, P], F32R)
            nc.scalar.dma_start(ones2[:], ones2_d[:].bitcast(F32R))
            wk_sb = w_pool.tile([P, NDIMC, EMB_C], BF16)
            nc.sync.dma_start(wk_sb[:], wk.rearrange("p (c e) -> p c e", c=NDIMC))
            bk_sb = w_pool.tile([P, NEMBC], F32)
            nc.sync.dma_start(bk_sb[:], bk[:])
            wq_sb = w_pool.tile([P, NDIMC, EMB_C], BF16)
            nc.sync.dma_start(wq_sb[:], wq.rearrange("p (c e) -> p c e", c=NDIMC))
            bq_sb = w_pool.tile([P, NEMBC], F32)
            nc.sync.dma_start(bq_sb[:], bq[:])
            wv_sb = w_pool.tile([P, NDIMC, EMB_C], BF16)
            nc.sync.dma_start(wv_sb[:], wv.rearrange("p (c e) -> p c e", c=NDIMC))
            bv_sb = w_pool.tile([P, NEMBC], F32)
            nc.sync.dma_start(bv_sb[:], bv[:])
            wo_sb = w_pool.tile([P, NEMBC, DIM], BF16)
            nc.sync.dma_start(wo_sb[:], wo.rearrange("p (c n) -> p c n", c=NEMBC))

            # ---- PE warm-up: junk matmuls on a memset scratch (no DMA dep)
            # so HAM unthrottles while the transposes stream in ----
            scr = w_pool.tile([P, IBLK], BF16)
            nc.gpsimd.memset(scr[:], 1.0)
            for chain in range(2):
                junk = psum.tile([P, IBLK], F32, tag="po", bufs=2,
                                 name=f"junk_{chain}")
                for i in range(8):
                    nc.tensor.matmul(
                        junk[:], scr[:, :P], scr[:],
                        start=(i == 0), stop=(i == 7),
                    )

            # ---- x^T via DMA transpose (bf16 XBAR path) ----
            # NOTES (all HW-verified):
            #  - all XBAR transposes must share one queue — concurrent
            #    DMA-transposes on sync+scalar corrupt each other;
            #  - an F32R-typed DMA on the same queue poisons them too.
            xT = big_pool.tile([P, NDIMC, SEQ], BF16)
            for c in range(NDIMC):
                nc.sync.dma_start(xT[:, c, :], x[:, c * P:(c + 1) * P],
                                  transpose=True)

            QT = big_pool.tile([P, NEMBC, SEQ], BF16)
            KT = big_pool.tile([P, NEMBC, SEQ], BF16)
            VP = big_pool.tile([P, NSEQT, NH * (DH + 1)], BF16)
            outT = big_pool.tile([P, NEMBC, SEQ], BF16)

            for h in range(NH):
                nc.gpsimd.memset(VP[:, :, h * (DH + 1) + DH], 1.0)

            # ---- K^T and Q^T, full seq. Loop c outer / j-block inner so each
            # LDWEIGHTS serves 2 matmuls; halves pipeline through the two
            # s0 psum slots. ----
            def proj_half(dst, wsb, bsb, e, half, junk_after=None):
                pa = psum.tile([P, 2, IBLK], F32, tag="s0", bufs=2,
                               name=f"pk_{dst is QT}_{e}_{half}")
                for c in range(NDIMC):
                    for j in range(2):
                        jb = half * 2 + j
                        nc.tensor.matmul(
                            pa[:, j, :],
                            wsb[:, c, e * P:(e + 1) * P],
                            xT[:, c, jb * IBLK:(jb + 1) * IBLK],
                            start=(c == 0), stop=(c == NDIMC - 1),
                        )
                    if junk_after and c == 3:
                        junk_after()
                for j in range(2):
                    jb = half * 2 + j
                    nc.scalar.activation(
                        dst[:, e, jb * IBLK:(jb + 1) * IBLK], pa[:, j, :],
                        mybir.ActivationFunctionType.Identity,
                        bias=bsb[:, e:e + 1], scale=1.0,
                    )

            _junk_n = [2]

            def emit_junk():
                chain = _junk_n[0]
                _junk_n[0] += 1
                junk = psum.tile([P, IBLK], F32, tag="po", bufs=2,
                                 name=f"junk_{chain}")
                for i in range(8):
                    nc.tensor.matmul(
                        junk[:], scr[:, :P], scr[:],
                        start=(i == 0), stop=(i == 7),
                    )

            # junk chains woven through: the static scheduler packs them into
            # the x-transpose DMA-wait bubbles so HAM stays at K=8/8
            for e in range(NEMBC):
                for half in range(2):
                    proj_half(KT, wk_sb, bk_sb, e, half, junk_after=emit_junk)
            for e in range(NEMBC):
                for half in range(2):
                    proj_half(QT, wq_sb, bq_sb, e, half,
                              junk_after=emit_junk if e == 0 else None)

            # ---- V' (one seq-tile) ----
            def emit_vp(s):
                pv = psum.tile([P, IBLK], F32, tag="po", bufs=2,
                               name=f"pv_{s}")
                for c in range(NDIMC):
                    nc.tensor.matmul(
                        pv[:, :EMB_C],
                        xT[:, c, s * P:(s + 1) * P],
                        wv_sb[:, c, :],
                        start=(c == 0), stop=(c == NDIMC - 1),
                    )
                nc.scalar.copy(
                    VP[:, s, :].rearrange("p (h x) -> p h x", h=NH)[:, :, :DH],
                    pv[:, :EMB_C].rearrange("p (h d) -> p h d", h=NH),
                )

            # ---- attention ----
            def emit_spair(ib, jt, hp):
                i0 = ib * IBLK
                ps = psum.tile([P, 2, IBLK], F32, tag="s0", bufs=2,
                               name=f"ps{hp}_{ib}_{jt}")
                for hh in range(2):
                    lo = hh * DH
                    nc.tensor.matmul(
                        ps[:, hh, :],
                        KT[lo:lo + DH, hp, jt * P:(jt + 1) * P],
                        QT[lo:lo + DH, hp, i0:i0 + IBLK],
                        start=True, stop=True,
                    )
                es = es_pool.tile([P, 2, IBLK], BF16, tag="es", bufs=4,
                                  name=f"es{hp}_{ib}_{jt}")
                if jt in ACT_JTS:
                    nc.scalar.activation(
                        es[:], ps[:], mybir.ActivationFunctionType.Exp,
                        bias=0.0, scale=ACT_EXP_SCALE,
                    )
                else:
                    nc.vector._custom_dve(
                        exp_op, out=es[:], in0=ps[:],
                        s0=0.0, s1=EXP_C1, imm2=EXP_C2,
                    )
                return es

            def emit_av(pavs, es, jt, hp):
                for hh in range(2):
                    h = hp * 2 + hh
                    nc.tensor.matmul(
                        pavs[hh][:DH + 1, :],
                        VP[:, jt, h * (DH + 1):(h + 1) * (DH + 1)],
                        es[:, hh, :],
                        start=(jt == 0), stop=(jt == NJT - 1),
                    )

            def div_filler(ib, hp, pavs):
                def go(ib=ib, hp=hp, pavs=pavs):
                    i0 = ib * IBLK
                    for hh in range(2):
                        h = hp * 2 + hh
                        pavc = stage_pool.tile([DH, IBLK], F32, tag="pavc",
                                               bufs=4, name=f"pavc_{ib}_{hp}_{hh}")
                        nc.scalar.copy(pavc[:], pavs[hh][:DH, :])
                        den_row = stage_pool.tile([1, IBLK], F32R, tag="den",
                                                  bufs=4, name=f"den_{ib}_{hp}_{hh}")
                        nc.vector.tensor_copy(
                            den_row[:], pavs[hh][DH:DH + 1, :].bitcast(F32R))
                        recb = psum.tile([P, IBLK], F32, tag="po", bufs=2,
                                         name=f"recb_{ib}_{hp}_{hh}")
                        nc.tensor.matmul(recb[:DH, :], ones2[0:1, :DH],
                                         den_row[:], start=True, stop=True)
                        recb_sb = stage_pool.tile([DH, IBLK], F32, tag="recb",
                                                  bufs=4, name=f"recb_sb_{ib}_{hp}_{hh}")
                        nc.vector.reciprocal_approx_fast(recb_sb[:], recb[:DH, :])
                        otf = stage_pool.tile([DH, IBLK], F32, tag="otf",
                                              bufs=4, name=f"otf_{ib}_{hp}_{hh}")
                        nc.vector.tensor_tensor(
                            otf[:], pavc[:], recb_sb[:],
                            mybir.AluOpType.mult,
                        )
                        e_c, e_lo = divmod(h * DH, P)
                        # bias-add + bf16 cast on ACT (Pool's tensor ops are
                        # ~18x slower than DVE/ACT for this — measured)
                        nc.scalar.activation(
                            outT[e_lo:e_lo + DH, e_c, i0:i0 + IBLK], otf[:],
                            mybir.ActivationFunctionType.Identity,
                            bias=bv_sb[e_lo:e_lo + DH, e_c:e_c + 1], scale=1.0,
                        )
                return go

            def oproj_unit(s):
                def go(s=s):
                    pos = [psum.tile([P, IBLK], F32, tag="po", bufs=2,
                                     name=f"po_{s}_{nb}") for nb in range(2)]
                    for e in range(NEMBC):
                        for nb in range(2):
                            nc.tensor.matmul(
                                pos[nb][:],
                                outT[:, e, s * P:(s + 1) * P],
                                wo_sb[:, e, nb * IBLK:(nb + 1) * IBLK],
                                start=(e == 0), stop=(e == NEMBC - 1),
                            )
                    oc = stage_pool.tile([P, DIM], BF16, tag="oc",
                                         bufs=2, name=f"oc_{s}")
                    for nb in range(2):
                        if (s + nb) % 2 == 0:
                            nc.scalar.copy(oc[:, nb * IBLK:(nb + 1) * IBLK],
                                           pos[nb][:])
                        else:
                            nc.vector.tensor_copy(
                                oc[:, nb * IBLK:(nb + 1) * IBLK], pos[nb][:])
                    nc.sync.dma_start(out[s * P:(s + 1) * P, :], oc[:])
                return go

            # Software-pipelined attention: one flat stream over all
            # (pass, jt) slots; AV trails S/exp by LAG slots so there is no
            # pass-boundary bubble on PE (regular bubbles resonate with the
            # 3.4us HAM window and throttle the PE clock — measured 83us of
            # K=4/8 time with the per-pass loop).
            # Ordering invariants (Tile deps are program-order only):
            #  - div(p-1) pops right BEFORE AV(p, 0): the new AVs reuse the
            #    pav psum slots the divide still reads;
            #  - oproj(ib) pops after both divs of ib (FIFO preserves this).
            POP_SLOTS = frozenset((4, 6, 8, 10, 12, 14))
            LAG = 2
            passes = [(ib, hp) for ib in range(NIBLK) for hp in range(2)]
            div_q = []
            work_q = []
            pav_of = {}
            es_of = {}
            total = len(passes) * NJT
            for t in range(total + LAG):
                if t < total:
                    p, jt = divmod(t, NJT)
                    ib, hp = passes[p]
                    es_of[t] = emit_spair(ib, jt, hp)
                    if p == 0:
                        emit_vp(jt)
                if t % NJT == LAG and div_q:
                    div_q.pop(0)()
                ta = t - LAG
                if ta >= 0 and ta < total:
                    pa, jta = divmod(ta, NJT)
                    iba, hpa = passes[pa]
                    if jta == 0:
                        pav_of[pa] = [
                            psum.tile([P, IBLK], F32, tag="pav", bufs=2,
                                      name=f"pav_{pa}_{hh}")
                            for hh in range(2)
                        ]
                    emit_av(pav_of[pa], es_of.pop(ta), jta, hpa)
                    if jta == NJT - 1:
                        div_q.append(div_filler(iba, hpa, pav_of.pop(pa)))
                        if hpa == 1:
                            for s in range(iba * (IBLK // P),
                                           (iba + 1) * (IBLK // P)):
                                work_q.append(oproj_unit(s))
                if t % NJT in POP_SLOTS and work_q:
                    work_q.pop(0)()

            for go in div_q:
                go()
            for go in work_q:
                go()

    nc.compile()
    return nc


def shard_inputs(inputs):
    """Full inputs dict -> list of 8 per-core input dicts."""
    import ml_dtypes
    BF = ml_dtypes.bfloat16
    x = np.asarray(inputs["x"], np.float32)
    x_bf = [np.ascontiguousarray(x[bi]).astype(BF) for bi in range(2)]
    wq = np.asarray(inputs["wq"], np.float32)
    wk = np.asarray(inputs["wk"], np.float32)
    wv = np.asarray(inputs["wv"], np.float32)
    wo = np.asarray(inputs["wo"], np.float32)
    ones2 = np.zeros((2, P), np.float32)
    ones2[0, :DH] = 1.0
    ones2[1, DH:] = 1.0
    def wT(m):  # [DIM, EMB_C] -> [P, NDIMC*EMB_C] (p-major chunks)
        return np.ascontiguousarray(
            m.reshape(NDIMC, P, EMB_C).transpose(1, 0, 2).reshape(P, -1)
        ).astype(BF)

    def bT(v):  # [EMB_C] -> [P, NEMBC]
        return np.ascontiguousarray(
            v.reshape(NEMBC, P).T.astype(np.float32))

    maps = []
    for core in range(8):
        bi, hg = divmod(core, 4)
        sl = slice(hg * EMB_C, (hg + 1) * EMB_C)
        wo_c = np.ascontiguousarray(wo[sl, :])
        maps.append({
            "x": x_bf[bi],
            "wq": wT(np.ascontiguousarray(wq[:, sl])),
            "wk": wT(np.ascontiguousarray(wk[:, sl] / 128.0)),
            "wv": wT(np.ascontiguousarray(wv[:, sl])),
            "bq": bT(inputs["bq"][sl]),
            "bk": bT(inputs["bk"][sl] / 128.0),
            "bv": bT(inputs["bv"][sl]),
            "wo": np.ascontiguousarray(
                wo_c.reshape(NEMBC, P, DIM).transpose(1, 0, 2).reshape(P, -1)
            ).astype(BF),
            "ones2": ones2,
        })
    return maps


def gather_outputs(results, bo):
    out = np.zeros((2, SEQ, DIM), np.float32)
    for core in range(8):
        bi = core // 4
        out[bi] += np.asarray(results[core]["out"]).astype(np.float32)
    out += np.asarray(bo, np.float32)
    return out


_NC_CACHE = {}


def _get_nc(row_pack=True):
    if row_pack not in _NC_CACHE:
        _NC_CACHE[row_pack] = build_kernel(row_pack=row_pack)
    return _NC_CACHE[row_pack]


def run_sharded(inputs, trace=False, row_pack=True):
    """Returns (full_output [2,2048,1024] fp32, BassKernelResults)."""
    from concourse import bass_utils
    nc = _get_nc(row_pack)
    maps = shard_inputs(inputs)
    res = bass_utils.run_bass_kernel_spmd(
        nc, maps, core_ids=list(range(8)), trace=trace,
    )
    out = gather_outputs(res.results, np.asarray(inputs["bo"]))
    return out, res


def kernel(**inputs):
    out, _ = run_sharded(inputs)
    return out


# revision 31
# speedup vs baseline: 1.3400x; 1.0184x over previous
"""Bass/Tile kernel for nn_Attention_81690277970645 on TRN2 (v2).

Sharding: 8 cores = 2 batches x 4 head-groups (4 heads of d=64 each).
Per core (batch bi, head-group hg):
  inputs:  x_b [2048, 1024] bf16, wq/wv slices [1024, 256] bf16,
           wk slice pre-scaled by 1/128 bf16, bq/bv [256] f32,
           bk/128 [256] f32, wo slice [256, 1024] bf16
  output:  partial out [2048, 1024] bf16 (host sums the 4 head-group
           partials per batch in fp32 and adds bo)

v2 changes vs baseline (271us):
  - x^T comes straight from a DMA-transpose (XBAR) of bf16 x: no PE
    transposes, no psum drains for x^T.
  - softmax exp is split between ACT (native Exp, scale=16 on the
    1/128-prescaled scores) and DVE (custom 8-stage op computing
    (p2(t))^16 with t = S/128, rel err ~1.4e-3 RMS) so neither engine
    gates the S->exp->AV pipeline.
  - all QKV/O weights bf16 (FWL on LDWEIGHTS), out written bf16.
  - PE warm-up junk-MM chain at t=0 so QKV projections run at 2.4GHz.
  - LDW-frugal output projection (stationary outT bf16), per-seq-tile
    [128,1024] output DMAs.
  - Pool (gpsimd) does the SBUF-side divide tail; reciprocal runs on
    [2,512] packed denominators, broadcast via one K=2 matmul.
"""
import sys
import numpy as np

if '/opt/trn_rl_repo' not in sys.path:
    sys.path.insert(0, '/opt/trn_rl_repo')

import concourse.mybir as mybir
from concourse import bacc
from concourse.tile import TileContext

F32 = mybir.dt.float32
F32R = mybir.dt.float32r
BF16 = mybir.dt.bfloat16

SEQ = 2048
DIM = 1024
EMB_C = 256          # per-core emb columns (4 heads x 64)
NH = 4               # heads per core
DH = 64
P = 128
NSEQT = SEQ // P     # 16 seq tiles
NDIMC = DIM // P     # 8 dim chunks
NEMBC = EMB_C // P   # 2 emb chunks
IBLK = 512
NIBLK = SEQ // IBLK  # 4 i-blocks
NJT = SEQ // P       # 16 j tiles

# exp(x) ~= (((EXP_C1*t + EXP_C2)*t + 1)^16 with t = x/16 = S_true/128
# (1/16 of the 1/8 softmax scale is folded into wk/bk host-side).
# Minimax fit over |x| <= 2.75: max rel 3.6e-3, weighted RMS 1.35e-3.
EXP_C1 = 0.49990254
EXP_C2 = 1.00360098
ACT_EXP_SCALE = 16.0
# jt slots whose exp runs on ACT (rest on DVE custom op)
ACT_JTS = frozenset((0, 2, 4, 6, 8, 9, 10, 12, 14))


def r(ap):
    return ap.bitcast(F32R)


_EXP_OP = None


def _register_exp_op():
    """Define + register the custom DVE op EXP_POLY16_ANT at runtime.

    The op registry in concourse.dve_ops only knows the stock ops; the
    per-NEFF table generator and CoreSim both dispatch by name, so the
    new op must appear in OPS / CUSTOM_DVE_SPECS / the sub-opcode map
    before nc.compile().
    """
    global _EXP_OP
    if _EXP_OP is not None:
        return _EXP_OP
    import concourse.dve_ops as dvo
    from concourse.dve_spec import Spec, Src0, C1, C2, One, lower
    from concourse.dve_uop import DveOpSpec

    name = "EXP_POLY16_ANT"
    for op in dvo.OPS:
        if op.name == name:
            _EXP_OP = op
            return op

    def sq(x):
        return x * x

    body = sq(sq(sq(sq((Src0 * C1 + C2) * Src0 + One))))

    def ref(in0, in1, s0, s1, imm2):
        t = in0.astype(np.float32)
        p = ((t * np.float32(s1) + np.float32(imm2)) * t
             + np.float32(1.0)).astype(np.float32)
        for _ in range(4):
            p = (p * p).astype(np.float32)
        return p

    spec = Spec(body=body, reference=ref)
    row = dvo._CUSTOM_DVE_ROW_BASE + len(dvo.OPS)
    dvo._SUB_OPCODE_FOR_NAME[name] = row
    shas = {}
    for ver in ("v3", "v4"):
        try:
            uops = lower(spec, ver=ver)
            shas[ver] = DveOpSpec(name=name, opcode=row, uops=uops,
                                  rd1_en=False).sha(ver)
        except Exception:
            pass
    op = dvo.DveOp(name, spec, subdim=False, uops_sha=shas)
    dvo.OPS.append(op)
    dvo.CUSTOM_DVE_SPECS[name] = op.spec
    _EXP_OP = op
    return op


def build_kernel(row_pack=True):
    exp_op = _register_exp_op()
    nc = bacc.Bacc("TRN2", target_bir_lowering=False, debug=False, num_devices=8)

    x = nc.dram_tensor("x", [SEQ, DIM], BF16, kind="ExternalInput")
    wq = nc.dram_tensor("wq", [P, NDIMC * EMB_C], BF16, kind="ExternalInput")
    wk = nc.dram_tensor("wk", [P, NDIMC * EMB_C], BF16, kind="ExternalInput")
    wv = nc.dram_tensor("wv", [P, NDIMC * EMB_C], BF16, kind="ExternalInput")
    bq = nc.dram_tensor("bq", [P, NEMBC], F32, kind="ExternalInput")
    bk = nc.dram_tensor("bk", [P, NEMBC], F32, kind="ExternalInput")
    bv = nc.dram_tensor("bv", [P, NEMBC], F32, kind="ExternalInput")
    wo = nc.dram_tensor("wo", [P, NEMBC * DIM], BF16, kind="ExternalInput")
    ones2_d = nc.dram_tensor("ones2", [2, P], F32, kind="ExternalInput")
    out = nc.dram_tensor("out", [SEQ, DIM], BF16, kind="ExternalOutput")

    with TileContext(nc) as tc:
        with (
            tc.tile_pool(name="w", bufs=1) as w_pool,
            tc.tile_pool(name="big", bufs=1) as big_pool,
            tc.tile_pool(name="stage", bufs=3) as stage_pool,
            tc.tile_pool(name="es", bufs=1) as es_pool,
            tc.tile_pool(name="ps", bufs=1, space="PSUM") as psum,
        ):
            # ---- constants + weights FIRST, all on the sync queue ahead of
            # the XBAR transposes: the scheduler chain-serializes every HWDGE
            # DMA against the transposes, and cross-queue that costs ~7us of
            # ping-pong per hop; same-queue the chain is free ----
            ones2 = w_pool.tile([2, P], F32R)
            nc.scalar.dma_start(ones2[:], ones2_d[:].bitcast(F32R))
            wk_sb = w_pool.tile([P, NDIMC, EMB_C], BF16)
            nc.sync.dma_start(wk_sb[:], wk.rearrange("p (c e) -> p c e", c=NDIMC))
            bk_sb = w_pool.tile([P, NEMBC], F32)
            nc.sync.dma_start(bk_sb[:], bk[:])
            wq_sb = w_pool.tile([P, NDIMC, EMB_C], BF16)
            nc.sync.dma_start(wq_sb[:], wq.rearrange("p (c e) -> p c e", c=NDIMC))
            bq_sb = w_pool.tile([P, NEMBC], F32)
            nc.sync.dma_start(bq_sb[:], bq[:])
            wv_sb = w_pool.tile([P, NDIMC, EMB_C], BF16)
            nc.sync.dma_start(wv_sb[:], wv.rearrange("p (c e) -> p c e", c=NDIMC))
            bv_sb = w_pool.tile([P, NEMBC], F32)
            nc.sync.dma_start(bv_sb[:], bv[:])
            wo_sb = w_pool.tile([P, NEMBC, DIM], BF16)
            nc.sync.dma_start(wo_sb[:], wo.rearrange("p (c n) -> p c n", c=NEMBC))

            # ---- PE warm-up: junk matmuls on a memset scratch (no DMA dep)
            # so HAM unthrottles while the transposes stream in ----
            scr = w_pool.tile([P, IBLK], BF16)
            nc.gpsimd.memset(scr[:], 1.0)
            for chain in range(2):
                junk = psum.tile([P, IBLK], F32, tag="po", bufs=2,
                                 name=f"junk_{chain}")
                for i in range(8):
                    nc.tensor.matmul(
                        junk[:], scr[:, :P], scr[:],
                        start=(i == 0), stop=(i == 7),
                    )

            # ---- x^T via DMA transpose (bf16 XBAR path) ----
            # NOTES (all HW-verified):
            #  - all XBAR transposes must share one queue — concurrent
            #    DMA-transposes on sync+scalar corrupt each other;
            #  - an F32R-typed DMA on the same queue poisons them too.
            xT = big_pool.tile([P, NDIMC, SEQ], BF16)
            for c in range(NDIMC):
                nc.sync.dma_start(xT[:, c, :], x[:, c * P:(c + 1) * P],
                                  transpose=True)

            QT = big_pool.tile([P, NEMBC, SEQ], BF16)
            KT = big_pool.tile([P, NEMBC, SEQ], BF16)
            VP = big_pool.tile([P, NSEQT, NH * (DH + 1)], BF16)
            outT = big_pool.tile([P, NEMBC, SEQ], BF16)

            for h in range(NH):
                nc.gpsimd.memset(VP[:, :, h * (DH + 1) + DH], 1.0)

            # ---- K^T and Q^T, full seq. Loop c outer / j-block inner so each
            # LDWEIGHTS serves 2 matmuls; halves pipeline through the two
            # s0 psum slots. ----
            def proj_half(dst, wsb, bsb, e, half, junk_after=None):
                pa = psum.tile([P, 2, IBLK], F32, tag="s0", bufs=2,
                               name=f"pk_{dst is QT}_{e}_{half}")
                for c in range(NDIMC):
                    for j in range(2):
                        jb = half * 2 + j
                        nc.tensor.matmul(
                            pa[:, j, :],
                            wsb[:, c, e * P:(e + 1) * P],
                            xT[:, c, jb * IBLK:(jb + 1) * IBLK],
                            start=(c == 0), stop=(c == NDIMC - 1),
                        )
                    if junk_after and c == 3:
                        junk_after()
                for j in range(2):
                    jb = half * 2 + j
                    nc.scalar.activation(
                        dst[:, e, jb * IBLK:(jb + 1) * IBLK], pa[:, j, :],
                        mybir.ActivationFunctionType.Identity,
                        bias=bsb[:, e:e + 1], scale=1.0,
                    )

            _junk_n = [2]

            def emit_junk():
                chain = _junk_n[0]
                _junk_n[0] += 1
                junk = psum.tile([P, IBLK], F32, tag="po", bufs=2,
                                 name=f"junk_{chain}")
                for i in range(8):
                    nc.tensor.matmul(
                        junk[:], scr[:, :P], scr[:],
                        start=(i == 0), stop=(i == 7),
                    )

            # junk chains woven through: the static scheduler packs them into
            # the x-transpose DMA-wait bubbles so HAM stays at K=8/8
            for e in range(NEMBC):
                for half in range(2):
                    proj_half(KT, wk_sb, bk_sb, e, half, junk_after=emit_junk)
            for e in range(NEMBC):
                for half in range(2):
                    proj_half(QT, wq_sb, bq_sb, e, half,
                              junk_after=emit_junk if e == 0 else None)

            # ---- V' (one seq-tile) ----
            def emit_vp(s):
                pv = psum.tile([P, IBLK], F32, tag="po", bufs=2,
                               name=f"pv_{s}")
                for c in range(NDIMC):
                    nc.tensor.matmul(
                        pv[:, :EMB_C],
                        xT[:, c, s * P:(s + 1) * P],
                        wv_sb[:, c, :],
                        start=(c == 0), stop=(c == NDIMC - 1),
                    )
                nc.scalar.copy(
                    VP[:, s, :].rearrange("p (h x) -> p h x", h=NH)[:, :, :DH],
                    pv[:, :EMB_C].rearrange("p (h d) -> p h d", h=NH),
                )

            # ---- attention ----
            def emit_spair(ib, jt, hp):
                i0 = ib * IBLK
                ps = psum.tile([P, 2, IBLK], F32, tag="s0", bufs=2,
                               name=f"ps{hp}_{ib}_{jt}")
                for hh in range(2):
                    lo = hh * DH
                    nc.tensor.matmul(
                        ps[:, hh, :],
                        KT[lo:lo + DH, hp, jt * P:(jt + 1) * P],
                        QT[lo:lo + DH, hp, i0:i0 + IBLK],
                        start=True, stop=True,
                    )
                es = es_pool.tile([P, 2, IBLK], BF16, tag="es", bufs=6,
                                  name=f"es{hp}_{ib}_{jt}")
                if jt in ACT_JTS:
                    nc.scalar.activation(
                        es[:], ps[:], mybir.ActivationFunctionType.Exp,
                        bias=0.0, scale=ACT_EXP_SCALE,
                    )
                else:
                    nc.vector._custom_dve(
                        exp_op, out=es[:], in0=ps[:],
                        s0=0.0, s1=EXP_C1, imm2=EXP_C2,
                    )
                return es

            def emit_av(pavs, es, jt, hp):
                for hh in range(2):
                    h = hp * 2 + hh
                    nc.tensor.matmul(
                        pavs[hh][:DH + 1, :],
                        VP[:, jt, h * (DH + 1):(h + 1) * (DH + 1)],
                        es[:, hh, :],
                        start=(jt == 0), stop=(jt == NJT - 1),
                    )

            def div_filler(ib, hp, pavs):
                def go(ib=ib, hp=hp, pavs=pavs):
                    i0 = ib * IBLK
                    for hh in range(2):
                        h = hp * 2 + hh
                        pavc = stage_pool.tile([DH, IBLK], F32, tag="pavc",
                                               bufs=4, name=f"pavc_{ib}_{hp}_{hh}")
                        nc.scalar.copy(pavc[:], pavs[hh][:DH, :])
                        den_row = stage_pool.tile([1, IBLK], F32R, tag="den",
                                                  bufs=4, name=f"den_{ib}_{hp}_{hh}")
                        nc.vector.tensor_copy(
                            den_row[:], pavs[hh][DH:DH + 1, :].bitcast(F32R))
                        recb = psum.tile([P, IBLK], F32, tag="po", bufs=2,
                                         name=f"recb_{ib}_{hp}_{hh}")
                        nc.tensor.matmul(recb[:DH, :], ones2[0:1, :DH],
                                         den_row[:], start=True, stop=True)
                        recb_sb = stage_pool.tile([DH, IBLK], F32, tag="recb",
                                                  bufs=4, name=f"recb_sb_{ib}_{hp}_{hh}")
                        nc.vector.reciprocal_approx_fast(recb_sb[:], recb[:DH, :])
                        otf = stage_pool.tile([DH, IBLK], F32, tag="otf",
                                              bufs=4, name=f"otf_{ib}_{hp}_{hh}")
                        nc.vector.tensor_tensor(
                            otf[:], pavc[:], recb_sb[:],
                            mybir.AluOpType.mult,
                        )
                        e_c, e_lo = divmod(h * DH, P)
                        # bias-add + bf16 cast on ACT (Pool's tensor ops are
                        # ~18x slower than DVE/ACT for this — measured)
                        nc.scalar.activation(
                            outT[e_lo:e_lo + DH, e_c, i0:i0 + IBLK], otf[:],
                            mybir.ActivationFunctionType.Identity,
                            bias=bv_sb[e_lo:e_lo + DH, e_c:e_c + 1], scale=1.0,
                        )
                return go

            def oproj_unit(s):
                def go(s=s):
                    pos = [psum.tile([P, IBLK], F32, tag="po", bufs=2,
                                     name=f"po_{s}_{nb}") for nb in range(2)]
                    for e in range(NEMBC):
                        for nb in range(2):
                            nc.tensor.matmul(
                                pos[nb][:],
                                outT[:, e, s * P:(s + 1) * P],
                                wo_sb[:, e, nb * IBLK:(nb + 1) * IBLK],
                                start=(e == 0), stop=(e == NEMBC - 1),
                            )
                    oc = stage_pool.tile([P, DIM], BF16, tag="oc",
                                         bufs=2, name=f"oc_{s}")
                    for nb in range(2):
                        if (s + nb) % 2 == 0:
                            nc.scalar.copy(oc[:, nb * IBLK:(nb + 1) * IBLK],
                                           pos[nb][:])
                        else:
                            nc.vector.tensor_copy(
                                oc[:, nb * IBLK:(nb + 1) * IBLK], pos[nb][:])
                        nc.sync.dma_start(
                            out[s * P:(s + 1) * P, nb * IBLK:(nb + 1) * IBLK],
                            oc[:, nb * IBLK:(nb + 1) * IBLK])
                return go

            # Software-pipelined attention: one flat stream over all
            # (pass, jt) slots; AV trails S/exp by LAG slots so there is no
            # pass-boundary bubble on PE (regular bubbles resonate with the
            # 3.4us HAM window and throttle the PE clock — measured 83us of
            # K=4/8 time with the per-pass loop).
            # Ordering invariants (Tile deps are program-order only):
            #  - div(p-1) pops right BEFORE AV(p, 0): the new AVs reuse the
            #    pav psum slots the divide still reads;
            #  - oproj(ib) pops after both divs of ib (FIFO preserves this).
            POP_SLOTS = frozenset((5, 13))
            LAG = 2
            passes = [(ib, hp) for ib in range(NIBLK) for hp in range(2)]
            div_q = []
            work_q = []
            pav_of = {}
            es_of = {}
            total = len(passes) * NJT
            for t in range(total + LAG):
                if t < total:
                    p, jt = divmod(t, NJT)
                    ib, hp = passes[p]
                    es_of[t] = emit_spair(ib, jt, hp)
                    if p == 0:
                        emit_vp(jt)
                if t % NJT == LAG and div_q:
                    div_q.pop(0)()
                ta = t - LAG
                if ta >= 0 and ta < total:
                    pa, jta = divmod(ta, NJT)
                    iba, hpa = passes[pa]
                    if jta == 0:
                        pav_of[pa] = [
                            psum.tile([P, IBLK], F32, tag="pav", bufs=2,
                                      name=f"pav_{pa}_{hh}")
                            for hh in range(2)
                        ]
                    emit_av(pav_of[pa], es_of.pop(ta), jta, hpa)
                    if jta == NJT - 1:
                        div_q.append(div_filler(iba, hpa, pav_of.pop(pa)))
                        if hpa == 1:
                            for s in range(iba * (IBLK // P),
                                           (iba + 1) * (IBLK // P)):
                                work_q.append(oproj_unit(s))
                if t % NJT in POP_SLOTS and work_q:
                    work_q.pop(0)()

            for go in div_q:
                go()
            for go in work_q:
                go()

    nc.compile()
    return nc


def shard_inputs(inputs):
    """Full inputs dict -> list of 8 per-core input dicts."""
    import ml_dtypes
    BF = ml_dtypes.bfloat16
    x = np.asarray(inputs["x"], np.float32)
    x_bf = [np.ascontiguousarray(x[bi]).astype(BF) for bi in range(2)]
    wq = np.asarray(inputs["wq"], np.float32)
    wk = np.asarray(inputs["wk"], np.float32)
    wv = np.asarray(inputs["wv"], np.float32)
    wo = np.asarray(inputs["wo"], np.float32)
    ones2 = np.zeros((2, P), np.float32)
    ones2[0, :DH] = 1.0
    ones2[1, DH:] = 1.0
    def wT(m):  # [DIM, EMB_C] -> [P, NDIMC*EMB_C] (p-major chunks)
        return np.ascontiguousarray(
            m.reshape(NDIMC, P, EMB_C).transpose(1, 0, 2).reshape(P, -1)
        ).astype(BF)

    def bT(v):  # [EMB_C] -> [P, NEMBC]
        return np.ascontiguousarray(
            v.reshape(NEMBC, P).T.astype(np.float32))

    maps = []
    for core in range(8):
        bi, hg = divmod(core, 4)
        sl = slice(hg * EMB_C, (hg + 1) * EMB_C)
        wo_c = np.ascontiguousarray(wo[sl, :])
        maps.append({
            "x": x_bf[bi],
            "wq": wT(np.ascontiguousarray(wq[:, sl])),
            "wk": wT(np.ascontiguousarray(wk[:, sl] / 128.0)),
            "wv": wT(np.ascontiguousarray(wv[:, sl])),
            "bq": bT(inputs["bq"][sl]),
            "bk": bT(inputs["bk"][sl] / 128.0),
            "bv": bT(inputs["bv"][sl]),
            "wo": np.ascontiguousarray(
                wo_c.reshape(NEMBC, P, DIM).transpose(1, 0, 2).reshape(P, -1)
            ).astype(BF),
            "ones2": ones2,
        })
    return maps


def gather_outputs(results, bo):
    out = np.zeros((2, SEQ, DIM), np.float32)
    for core in range(8):
        bi = core // 4
        out[bi] += np.asarray(results[core]["out"]).astype(np.float32)
    out += np.asarray(bo, np.float32)
    return out


_NC_CACHE = {}


def _get_nc(row_pack=True):
    if row_pack not in _NC_CACHE:
        _NC_CACHE[row_pack] = build_kernel(row_pack=row_pack)
    return _NC_CACHE[row_pack]


def run_sharded(inputs, trace=False, row_pack=True):
    """Returns (full_output [2,2048,1024] fp32, BassKernelResults)."""
    from concourse import bass_utils
    nc = _get_nc(row_pack)
    maps = shard_inputs(inputs)
    res = bass_utils.run_bass_kernel_spmd(
        nc, maps, core_ids=list(range(8)), trace=trace,
    )
    out = gather_outputs(res.results, np.asarray(inputs["bo"]))
    return out, res


def kernel(**inputs):
    out, _ = run_sharded(inputs)
    return out
